# revision 28
# baseline (speedup 1.0000x reference)
"""MoEBertSelfAttention on 8 Trainium2 NeuronCores.

Strategy: data-parallel over batch (B=8 -> one batch element per core).
Each core computes its element's full self-attention:
    q = h @ Wq.T + bq ; k, v likewise
    S = q_h k_h^T / sqrt(dh) + mask ; P = softmax(S) * head_mask
    ctx = P v_h, heads concatenated.

On-device dataflow is fully transposed to avoid any on-chip transposes:
  - host passes H^T and W^T; projections produce Q^T/K^T (feature-major)
    and V in normal layout (token-major),
  - scores are computed as S^T (key position on partitions) so the additive
    attention mask is a per-partition bias on the exp() activation,
  - the softmax denominator rides as an extra all-ones column of V in the
    PV matmul; normalization uses a batched reciprocal (reshaped to all 128
    partitions via a DRAM bounce) and a partition-broadcast DMA,
  - host transposes the returned ctx^T back.
head_mask is folded into Wv/bv on the host (exact: probs*hm @ V == probs @ (hm*V)).
Matmuls run in float32r (full PE rate); PSUM/softmax stay fp32.

The next head-pair's Q/K projection matmuls are software-pipelined into the
current pair's attention loop as PE filler work so the tensor engine never
idles while the activation engine runs exp().
"""

import sys

if "/opt/trn_rl_repo" not in sys.path:
    sys.path.insert(0, "/opt/trn_rl_repo")

import numpy as np

import concourse.bacc as bacc
import concourse.bass as bass
import concourse.tile as tile
from concourse import mybir
from concourse.bass_utils import run_bass_kernel_spmd

S = 1024  # sequence length
D = 1024  # hidden size
H = 16  # heads
DH = 64  # head size
KT = D // 128  # 128-row tiles along a feature dim
NT = S // 512  # 512-col tiles along the sequence
HP = H // 2  # head pairs
N_CORES = 8

F32 = mybir.dt.float32
F32R = mybir.dt.float32r


def _ts(i, n):
    return slice(i * n, (i + 1) * n)


def build_program():
    nc = bacc.Bacc("TRN2", target_bir_lowering=False, debug=False, num_devices=N_CORES)

    hT = nc.dram_tensor("hT", [D, S], F32R, kind="ExternalInput").ap()
    wqT = nc.dram_tensor("wqT", [D, D], F32R, kind="ExternalInput").ap()
    wkT = nc.dram_tensor("wkT", [D, D], F32R, kind="ExternalInput").ap()
    wvT = nc.dram_tensor("wvT", [D, D], F32R, kind="ExternalInput").ap()
    bq2d = nc.dram_tensor("bq2d", [128, KT], F32, kind="ExternalInput").ap()
    bk2d = nc.dram_tensor("bk2d", [128, KT], F32, kind="ExternalInput").ap()
    bvrow = nc.dram_tensor("bvrow", [1, D], F32, kind="ExternalInput").ap()
    mask2d = nc.dram_tensor("mask2d", [128, KT], F32, kind="ExternalInput").ap()
    ctxT = nc.dram_tensor("ctxT", [D, S], F32, kind="ExternalOutput").ap()
    # DRAM bounce buffers: rowsums out, reciprocals back (per head, flat 1024)
    rsums = nc.dram_tensor("rsums", [H, NT, 512], F32).ap()
    recips = nc.dram_tensor("recips", [H, NT, 512], F32).ap()

    hT_r = hT.rearrange("(kt p) s -> p kt s", p=128)
    wqT_r = wqT.rearrange("(kt p) o -> p kt o", p=128)
    wkT_r = wkT.rearrange("(kt p) o -> p kt o", p=128)
    wvT_r = wvT.rearrange("(kt p) o -> p kt o", p=128)

    with tile.TileContext(nc) as tc:
        with (
            tc.tile_pool(name="persist", bufs=1) as persist,
            tc.tile_pool(name="wpool", bufs=2) as wpool,
            tc.tile_pool(name="qkpool", bufs=2) as qkpool,
            tc.tile_pool(name="expool", bufs=8) as expool,
            tc.tile_pool(name="outpool", bufs=4) as outpool,
            tc.tile_pool(name="ps", bufs=2, space="PSUM") as ps,
        ):
            # ---- persistent SBUF ----
            # (first hT chunk + head pair 0's weights lead the DMA queues so
            # the first matmul can start within a few microseconds)
            hT_sb = persist.tile([128, KT, S], F32R)
            for kt in range(KT):
                nc.sync.dma_start(out=hT_sb[:, kt, :], in_=hT_r[:, kt, :])
            bq_sb = persist.tile([128, KT], F32)
            nc.sync.dma_start(out=bq_sb, in_=bq2d)
            bk_sb = persist.tile([128, KT], F32)
            nc.sync.dma_start(out=bk_sb, in_=bk2d)
            mask_sb = persist.tile([128, KT], F32)
            nc.sync.dma_start(out=mask_sb, in_=mask2d)
            # bv broadcast to all partitions (partition-step-0 DMA from DRAM)
            bv_bc = persist.tile([128, D], F32)
            nc.sync.dma_start(
                out=bv_bc,
                in_=bass.AP(tensor=bvrow.tensor, offset=0, ap=[[0, 128], [1, D]]),
            )
            ones_f = persist.tile([128, H], F32)
            nc.vector.memset(ones_f, 1.0)

            # V in token-major layout, one 65-wide block per head
            # ([64 cols of V_h | ones]); the ones column yields the softmax
            # denominator for free during the PV matmul.
            v_sb = persist.tile([128, KT, H * (DH + 1)], F32R)
            v4 = v_sb.rearrange("p st (h c) -> p st h c", c=DH + 1)
            for st in range(KT):
                nc.vector.tensor_copy(
                    v4[:, st, :, DH : DH + 1],
                    ones_f.rearrange("p (h o) -> p h o", o=1),
                )

            qk_tiles = {}

            def emit_qk(hp):
                """Q^T/K^T projection for head pair hp, yielded in small pieces
                so the caller can interleave them into attention emission."""
                wq_blk = wpool.tile([128, KT, 128], F32R, tag="wq", name=f"wq{hp}")
                nc.sync.dma_start(out=wq_blk, in_=wqT_r[:, :, _ts(hp, 128)])
                wk_blk = wpool.tile([128, KT, 128], F32R, tag="wk", name=f"wk{hp}")
                nc.sync.dma_start(out=wk_blk, in_=wkT_r[:, :, _ts(hp, 128)])
                res = []
                for pi, (blk, bias, tg) in enumerate(
                    ((wq_blk, bq_sb, "qT"), (wk_blk, bk_sb, "kT"))
                ):
                    t = qkpool.tile([128, S], F32R, tag=tg, name=f"{tg}{hp}")
                    for nt in range(NT):
                        p0 = ps.tile(
                            [128, 512], F32, tag="qk", bufs=1, name=f"pq{hp}_{pi}{nt}"
                        )
                        for kt in range(KT):
                            nc.tensor.matmul(
                                p0,
                                blk[:, kt, :],
                                hT_sb[:, kt, _ts(nt, 512)],
                                start=(kt == 0),
                                stop=(kt == KT - 1),
                            )
                            if kt % 2 == 1:
                                yield
                        nc.vector.tensor_scalar_add(
                            t[:, _ts(nt, 512)], p0, bias[:, hp : hp + 1]
                        )
                        yield
                    res.append(t)
                qk_tiles[hp] = res

            # head pair 0's projections up-front: weight DMAs + first matmuls
            # lead, then the remaining hT chunks, then the rest.
            for _ in emit_qk(0):
                pass

            # ---- V projection: V[s, o] = sum_d H^T[d, s] Wv^T[d, o] + bv[o] ----
            wvT_sb = persist.tile([128, KT, D], F32R)
            for kt in range(KT):
                nc.sync.dma_start(out=wvT_sb[:, kt, :], in_=wvT_r[:, kt, :])
            for st in range(KT):
                for nt in range(NT):
                    ps_v = ps.tile([128, 512], F32, tag="pv", bufs=3, name=f"psv{st}_{nt}")
                    for kt in range(KT):
                        nc.tensor.matmul(
                            ps_v,
                            hT_sb[:, kt, _ts(st, 128)],
                            wvT_sb[:, kt, _ts(nt, 512)],
                            start=(kt == 0),
                            stop=(kt == KT - 1),
                        )
                    # scatter into v_sb with the bias added on the way
                    nc.vector.tensor_tensor(
                        out=v4[:, st, 8 * nt : 8 * nt + 8, 0:DH],
                        in0=ps_v.rearrange("p (h c) -> p h c", c=DH),
                        in1=bv_bc[:, _ts(nt, 512)].rearrange("p (h c) -> p h c", c=DH),
                        op=mybir.AluOpType.add,
                    )

            # ---- attention, one-deep software pipeline over (pair, head, mt):
            # PV matmuls for unit n are emitted after unit n+1's scores+exp so
            # they never head-of-line-block the PE queue while exp(n) runs.
            pv_tiles = {}

            def emit_pv(hp, hl, mt, ex):
                h = 2 * hp + hl
                if mt == 0:
                    pv_tiles[h] = [
                        ps.tile(
                            [DH + 1, 512], F32, tag="pv", bufs=3, name=f"pspv{h}_{i}"
                        )
                        for i in range(NT)
                    ]
                for nt in range(NT):
                    nc.tensor.matmul(
                        pv_tiles[h][nt],
                        v_sb[:, mt, h * (DH + 1) : (h + 1) * (DH + 1)],
                        ex[:, _ts(nt, 512)],
                        start=(mt == 0),
                        stop=(mt == KT - 1),
                    )
                if mt == KT - 1:
                    emit_norm(h)

            def emit_norm(h):
                # rowsum row -> DRAM, batched reciprocal on [128, 8] (all
                # lanes), back to DRAM, partition-broadcast loads, final mul.
                ps_pv = pv_tiles[h]
                # copy ctx + rowsum out of PSUM first so the banks free quickly
                rs_sb = outpool.tile([DH + 1, NT, 512], F32, tag="rs", bufs=2, name=f"rs{h}")
                cs_sb = outpool.tile([DH, NT, 512], F32, tag="cs", bufs=2, name=f"cs{h}")
                for nt in range(NT):
                    nc.vector.tensor_copy(
                        rs_sb[DH : DH + 1, nt, :], ps_pv[nt][DH : DH + 1, :]
                    )
                    nc.vector.tensor_copy(cs_sb[:, nt, :], ps_pv[nt][0:DH, :])
                    nc.sync.dma_start(out=rsums[h, nt, :], in_=rs_sb[DH : DH + 1, nt, :])
                rc_sb = outpool.tile([128, KT], F32, tag="rc", bufs=2, name=f"rc{h}")
                nc.sync.dma_start(
                    out=rc_sb,
                    in_=bass.AP(tensor=rsums.tensor, offset=h * S, ap=[[KT, 128], [1, KT]]),
                )
                nc.vector.reciprocal(rc_sb, rc_sb)
                nc.sync.dma_start(
                    out=bass.AP(
                        tensor=recips.tensor, offset=h * S, ap=[[KT, 128], [1, KT]]
                    ),
                    in_=rc_sb,
                )
                for nt in range(NT):
                    bc_t = outpool.tile([DH, 512], F32, tag="bc", name=f"bc{h}_{nt}")
                    nc.sync.dma_start(
                        out=bc_t,
                        in_=bass.AP(
                            tensor=recips.tensor,
                            offset=h * S + nt * 512,
                            ap=[[0, DH], [1, 512]],
                        ),
                    )
                    stage = outpool.tile([DH, 512], F32, tag="stage", name=f"st{h}_{nt}")
                    nc.vector.tensor_mul(stage, cs_sb[:, nt, :], bc_t)
                    nc.sync.dma_start(
                        out=ctxT[h * DH : (h + 1) * DH, _ts(nt, 512)], in_=stage
                    )

            pending_pv = None
            for hp in range(HP):
                qT_t, kT_t = qk_tiles[hp]
                nxt = emit_qk(hp + 1) if hp + 1 < HP else iter(())
                for hl in range(2):
                    h = 2 * hp + hl
                    base = 64 * hl
                    for mt in range(KT):
                        # S^T[kpos, q] for this head
                        ps_s = ps.tile([128, 1024], F32, tag="sc", name=f"pss{h}_{mt}")
                        for nt in range(NT):
                            nc.tensor.matmul(
                                ps_s[:, _ts(nt, 512)],
                                kT_t[base : base + 64, _ts(mt, 128)],
                                qT_t[base : base + 64, _ts(nt, 512)],
                                start=True,
                                stop=True,
                            )
                        # probs_unnorm = exp(S^T/8 + mask[kpos])
                        ex = expool.tile([128, S], F32R, tag="ex", name=f"ex{h}_{mt}")
                        nc.scalar.activation(
                            ex,
                            ps_s,
                            mybir.ActivationFunctionType.Exp,
                            bias=mask_sb[:, mt : mt + 1],
                            scale=0.125,
                        )
                        if pending_pv is not None:
                            emit_pv(*pending_pv)
                        pending_pv = (hp, hl, mt, ex)
                        next(nxt, None)
                        next(nxt, None)
                # flush any remaining pipelined projection work
                for _ in nxt:
                    pass
            if pending_pv is not None:
                emit_pv(*pending_pv)
    nc.compile()
    return nc


_NC_CACHE = None


def _get_nc():
    global _NC_CACHE
    if _NC_CACHE is None:
        _NC_CACHE = build_program()
    return _NC_CACHE


def _prep_inputs(hidden_states, attention_mask, head_mask, Wq, bq, Wk, bk, Wv, bv):
    hidden_states = np.asarray(hidden_states, dtype=np.float32)
    attention_mask = np.asarray(attention_mask, dtype=np.float32)
    head_mask = np.asarray(head_mask, dtype=np.float32)
    Wq = np.asarray(Wq, dtype=np.float32)
    bq = np.asarray(bq, dtype=np.float32)
    Wk = np.asarray(Wk, dtype=np.float32)
    bk = np.asarray(bk, dtype=np.float32)
    Wv = np.asarray(Wv, dtype=np.float32)
    bv = np.asarray(bv, dtype=np.float32)

    # fold head_mask into Wv/bv (probs*hm @ V == probs @ (hm*V))
    hm = head_mask.reshape(H)
    hscale = np.repeat(hm, DH).astype(np.float32)
    wqT = np.ascontiguousarray(Wq.T)
    wkT = np.ascontiguousarray(Wk.T)
    wvT = np.ascontiguousarray((Wv * hscale[:, None]).T)
    bq2d = np.ascontiguousarray(bq.reshape(KT, 128).T)
    bk2d = np.ascontiguousarray(bk.reshape(KT, 128).T)
    bvrow = (bv * hscale).reshape(1, D)

    mask = np.broadcast_to(
        attention_mask.reshape(attention_mask.shape[0], -1)[:, -S:], (N_CORES, S)
    )

    in_maps = []
    for b in range(N_CORES):
        in_maps.append(
            {
                "hT": np.ascontiguousarray(hidden_states[b].T),
                "wqT": wqT,
                "wkT": wkT,
                "wvT": wvT,
                "bq2d": bq2d,
                "bk2d": bk2d,
                "bvrow": bvrow,
                "mask2d": np.ascontiguousarray(mask[b].reshape(KT, 128).T),
            }
        )
    return in_maps


def _install_trace_shim():
    """antenv.axon_hooks is absent in this image; provide it so trace=True works."""
    import types

    if "antenv.axon_hooks" in sys.modules:
        return
    mod = types.ModuleType("antenv.axon_hooks")
    mod._hook = None

    def _set(h):
        mod._hook = h

    def _get():
        return mod._hook

    mod.set_axon_ntff_profile_hook = _set
    mod.get_axon_ntff_profile_hook = _get
    sys.modules["antenv.axon_hooks"] = mod
    try:
        from trn_agent_boot.trn_boot import _ntff_profile_via_ctypes

        _set(_ntff_profile_via_ctypes("/opt/axon/libaxon_pjrt.so"))
    except Exception:
        pass


def _kernel_impl(trace=False, **inputs):
    nc = _get_nc()
    in_maps = _prep_inputs(**inputs)
    kwargs = {}
    if trace:
        _install_trace_shim()
        kwargs["trace"] = True
        kwargs["trace_cores"] = list(range(N_CORES))
    res = run_bass_kernel_spmd(nc, in_maps, core_ids=list(range(N_CORES)), **kwargs)
    out = np.empty((N_CORES, S, D), dtype=np.float32)
    for b in range(N_CORES):
        out[b] = res.results[b]["ctxT"].T
    return out, res


def kernel(**inputs) -> np.ndarray:
    return _kernel_impl(trace=False, **inputs)[0]


# revision 30
# speedup vs baseline: 1.0127x; 1.0127x over previous
"""MoEBertSelfAttention on 8 Trainium2 NeuronCores.

Strategy: data-parallel over batch (B=8 -> one batch element per core).
Each core computes its element's full self-attention:
    q = h @ Wq.T + bq ; k, v likewise
    S = q_h k_h^T / sqrt(dh) + mask ; P = softmax(S) * head_mask
    ctx = P v_h, heads concatenated.

On-device dataflow is fully transposed to avoid any on-chip transposes:
  - host passes H^T and W^T; projections produce Q^T/K^T (feature-major)
    and V in normal layout (token-major),
  - scores are computed as S^T (key position on partitions) so the additive
    attention mask is a per-partition bias on the exp() activation,
  - the softmax denominator rides as an extra all-ones column of V in the
    PV matmul; normalization uses a batched reciprocal (reshaped to all 128
    partitions via a DRAM bounce) and a partition-broadcast DMA,
  - host transposes the returned ctx^T back.
head_mask is folded into Wv/bv on the host (exact: probs*hm @ V == probs @ (hm*V)).
Matmuls run in float32r (full PE rate); PSUM/softmax stay fp32.

The next head-pair's Q/K projection matmuls are software-pipelined into the
current pair's attention loop as PE filler work so the tensor engine never
idles while the activation engine runs exp().
"""

import sys

if "/opt/trn_rl_repo" not in sys.path:
    sys.path.insert(0, "/opt/trn_rl_repo")

import numpy as np

import concourse.bacc as bacc
import concourse.bass as bass
import concourse.tile as tile
from concourse import mybir
from concourse.bass_utils import run_bass_kernel_spmd

S = 1024  # sequence length
D = 1024  # hidden size
H = 16  # heads
DH = 64  # head size
KT = D // 128  # 128-row tiles along a feature dim
NT = S // 512  # 512-col tiles along the sequence
HP = H // 2  # head pairs
N_CORES = 8

F32 = mybir.dt.float32
F32R = mybir.dt.float32r


def _ts(i, n):
    return slice(i * n, (i + 1) * n)


def build_program():
    nc = bacc.Bacc("TRN2", target_bir_lowering=False, debug=False, num_devices=N_CORES)

    hT = nc.dram_tensor("hT", [D, S], F32R, kind="ExternalInput").ap()
    wqT = nc.dram_tensor("wqT", [D, D], F32R, kind="ExternalInput").ap()
    wkT = nc.dram_tensor("wkT", [D, D], F32R, kind="ExternalInput").ap()
    wvT = nc.dram_tensor("wvT", [D, D], F32R, kind="ExternalInput").ap()
    bq2d = nc.dram_tensor("bq2d", [128, KT], F32, kind="ExternalInput").ap()
    bk2d = nc.dram_tensor("bk2d", [128, KT], F32, kind="ExternalInput").ap()
    bvrow = nc.dram_tensor("bvrow", [1, D], F32, kind="ExternalInput").ap()
    mask2d = nc.dram_tensor("mask2d", [128, KT], F32, kind="ExternalInput").ap()
    ctxT = nc.dram_tensor("ctxT", [D, S], F32, kind="ExternalOutput").ap()
    # DRAM bounce buffers: rowsums out, reciprocals back (per head, flat 1024)
    rsums = nc.dram_tensor("rsums", [H, NT, 512], F32).ap()
    recips = nc.dram_tensor("recips", [H, NT, 512], F32).ap()

    hT_r = hT.rearrange("(kt p) s -> p kt s", p=128)
    wqT_r = wqT.rearrange("(kt p) o -> p kt o", p=128)
    wkT_r = wkT.rearrange("(kt p) o -> p kt o", p=128)
    wvT_r = wvT.rearrange("(kt p) o -> p kt o", p=128)

    with tile.TileContext(nc) as tc:
        with (
            tc.tile_pool(name="persist", bufs=1) as persist,
            tc.tile_pool(name="wpool", bufs=2) as wpool,
            tc.tile_pool(name="qkpool", bufs=2) as qkpool,
            tc.tile_pool(name="expool", bufs=8) as expool,
            tc.tile_pool(name="outpool", bufs=4) as outpool,
            tc.tile_pool(name="ps", bufs=2, space="PSUM") as ps,
        ):
            # ---- persistent SBUF ----
            # (first hT chunk + head pair 0's weights lead the DMA queues so
            # the first matmul can start within a few microseconds)
            hT_sb = persist.tile([128, KT, S], F32R)
            for kt in range(KT):
                for hh in range(2):
                    nc.sync.dma_start(
                        out=hT_sb[:, kt, _ts(hh, 512)], in_=hT_r[:, kt, _ts(hh, 512)]
                    )
            bq_sb = persist.tile([128, KT], F32)
            nc.sync.dma_start(out=bq_sb, in_=bq2d)
            bk_sb = persist.tile([128, KT], F32)
            nc.sync.dma_start(out=bk_sb, in_=bk2d)
            mask_sb = persist.tile([128, KT], F32)
            nc.sync.dma_start(out=mask_sb, in_=mask2d)
            # bv broadcast to all partitions (partition-step-0 DMA from DRAM)
            bv_bc = persist.tile([128, D], F32)
            nc.sync.dma_start(
                out=bv_bc,
                in_=bass.AP(tensor=bvrow.tensor, offset=0, ap=[[0, 128], [1, D]]),
            )
            ones_f = persist.tile([128, H], F32)
            nc.vector.memset(ones_f, 1.0)

            # V in token-major layout, one 65-wide block per head
            # ([64 cols of V_h | ones]); the ones column yields the softmax
            # denominator for free during the PV matmul.
            v_sb = persist.tile([128, KT, H * (DH + 1)], F32R)
            v4 = v_sb.rearrange("p st (h c) -> p st h c", c=DH + 1)
            for st in range(KT):
                nc.vector.tensor_copy(
                    v4[:, st, :, DH : DH + 1],
                    ones_f.rearrange("p (h o) -> p h o", o=1),
                )

            qk_tiles = {}

            def emit_qk(hp):
                """Q^T/K^T projection for head pair hp, yielded in small pieces
                so the caller can interleave them into attention emission."""
                wq_blk = wpool.tile([128, KT, 128], F32R, tag="wq", name=f"wq{hp}")
                nc.sync.dma_start(out=wq_blk, in_=wqT_r[:, :, _ts(hp, 128)])
                wk_blk = wpool.tile([128, KT, 128], F32R, tag="wk", name=f"wk{hp}")
                nc.sync.dma_start(out=wk_blk, in_=wkT_r[:, :, _ts(hp, 128)])
                res = []
                for pi, (blk, bias, tg) in enumerate(
                    ((wq_blk, bq_sb, "qT"), (wk_blk, bk_sb, "kT"))
                ):
                    t = qkpool.tile([128, S], F32R, tag=tg, name=f"{tg}{hp}")
                    for nt in range(NT):
                        p0 = ps.tile(
                            [128, 512], F32, tag="qk", bufs=1, name=f"pq{hp}_{pi}{nt}"
                        )
                        for kt in range(KT):
                            nc.tensor.matmul(
                                p0,
                                blk[:, kt, :],
                                hT_sb[:, kt, _ts(nt, 512)],
                                start=(kt == 0),
                                stop=(kt == KT - 1),
                            )
                            if kt % 2 == 1:
                                yield
                        nc.vector.tensor_scalar_add(
                            t[:, _ts(nt, 512)], p0, bias[:, hp : hp + 1]
                        )
                        yield
                    res.append(t)
                qk_tiles[hp] = res

            # head pair 0's projections up-front: weight DMAs + first matmuls
            # lead, then the remaining hT chunks, then the rest.
            for _ in emit_qk(0):
                pass

            # ---- V projection: V[s, o] = sum_d H^T[d, s] Wv^T[d, o] + bv[o] ----
            wvT_sb = persist.tile([128, KT, D], F32R)
            for kt in range(KT):
                for hh in range(2):
                    nc.sync.dma_start(
                        out=wvT_sb[:, kt, _ts(hh, 512)], in_=wvT_r[:, kt, _ts(hh, 512)]
                    )
            for st in range(KT):
                for nt in range(NT):
                    ps_v = ps.tile([128, 512], F32, tag="pv", bufs=3, name=f"psv{st}_{nt}")
                    for kt in range(KT):
                        nc.tensor.matmul(
                            ps_v,
                            hT_sb[:, kt, _ts(st, 128)],
                            wvT_sb[:, kt, _ts(nt, 512)],
                            start=(kt == 0),
                            stop=(kt == KT - 1),
                        )
                    # scatter into v_sb with the bias added on the way
                    nc.vector.tensor_tensor(
                        out=v4[:, st, 8 * nt : 8 * nt + 8, 0:DH],
                        in0=ps_v.rearrange("p (h c) -> p h c", c=DH),
                        in1=bv_bc[:, _ts(nt, 512)].rearrange("p (h c) -> p h c", c=DH),
                        op=mybir.AluOpType.add,
                    )

            # ---- attention, one-deep software pipeline over (pair, head, mt):
            # PV matmuls for unit n are emitted after unit n+1's scores+exp so
            # they never head-of-line-block the PE queue while exp(n) runs.
            pv_tiles = {}

            def emit_pv(hp, hl, mt, ex):
                h = 2 * hp + hl
                if mt == 0:
                    pv_tiles[h] = [
                        ps.tile(
                            [DH + 1, 512], F32, tag="pv", bufs=3, name=f"pspv{h}_{i}"
                        )
                        for i in range(NT)
                    ]
                for nt in range(NT):
                    nc.tensor.matmul(
                        pv_tiles[h][nt],
                        v_sb[:, mt, h * (DH + 1) : (h + 1) * (DH + 1)],
                        ex[:, _ts(nt, 512)],
                        start=(mt == 0),
                        stop=(mt == KT - 1),
                    )
                if mt == KT - 1:
                    emit_norm(h)

            def emit_norm(h):
                # rowsum row -> DRAM, batched reciprocal on [128, 8] (all
                # lanes), back to DRAM, partition-broadcast loads, final mul.
                ps_pv = pv_tiles[h]
                # copy ctx + rowsum out of PSUM first so the banks free quickly
                rs_sb = outpool.tile([DH + 1, NT, 512], F32, tag="rs", bufs=2, name=f"rs{h}")
                cs_sb = outpool.tile([DH, NT, 512], F32, tag="cs", bufs=2, name=f"cs{h}")
                for nt in range(NT):
                    nc.vector.tensor_copy(
                        rs_sb[DH : DH + 1, nt, :], ps_pv[nt][DH : DH + 1, :]
                    )
                    nc.vector.tensor_copy(cs_sb[:, nt, :], ps_pv[nt][0:DH, :])
                    nc.sync.dma_start(out=rsums[h, nt, :], in_=rs_sb[DH : DH + 1, nt, :])
                rc_sb = outpool.tile([128, KT], F32, tag="rc", bufs=2, name=f"rc{h}")
                nc.sync.dma_start(
                    out=rc_sb,
                    in_=bass.AP(tensor=rsums.tensor, offset=h * S, ap=[[KT, 128], [1, KT]]),
                )
                nc.vector.reciprocal(rc_sb, rc_sb)
                nc.sync.dma_start(
                    out=bass.AP(
                        tensor=recips.tensor, offset=h * S, ap=[[KT, 128], [1, KT]]
                    ),
                    in_=rc_sb,
                )
                for nt in range(NT):
                    bc_t = outpool.tile([DH, 512], F32, tag="bc", name=f"bc{h}_{nt}")
                    nc.sync.dma_start(
                        out=bc_t,
                        in_=bass.AP(
                            tensor=recips.tensor,
                            offset=h * S + nt * 512,
                            ap=[[0, DH], [1, 512]],
                        ),
                    )
                    stage = outpool.tile([DH, 512], F32, tag="stage", name=f"st{h}_{nt}")
                    nc.vector.tensor_mul(stage, cs_sb[:, nt, :], bc_t)
                    nc.sync.dma_start(
                        out=ctxT[h * DH : (h + 1) * DH, _ts(nt, 512)], in_=stage
                    )

            pending_pv = []
            for hp in range(HP):
                qT_t, kT_t = qk_tiles[hp]
                nxt = emit_qk(hp + 1) if hp + 1 < HP else iter(())
                for hl in range(2):
                    h = 2 * hp + hl
                    base = 64 * hl
                    for mt in range(KT):
                        # S^T[kpos, q] for this head
                        ps_s = ps.tile([128, 1024], F32, tag="sc", name=f"pss{h}_{mt}")
                        for nt in range(NT):
                            nc.tensor.matmul(
                                ps_s[:, _ts(nt, 512)],
                                kT_t[base : base + 64, _ts(mt, 128)],
                                qT_t[base : base + 64, _ts(nt, 512)],
                                start=True,
                                stop=True,
                            )
                        # probs_unnorm = exp(S^T/8 + mask[kpos])
                        ex = expool.tile([128, S], F32R, tag="ex", name=f"ex{h}_{mt}")
                        nc.scalar.activation(
                            ex,
                            ps_s,
                            mybir.ActivationFunctionType.Exp,
                            bias=mask_sb[:, mt : mt + 1],
                            scale=0.125,
                        )
                        pending_pv.append((hp, hl, mt, ex))
                        depth = 1 if (hp == HP - 1 and hl == 1) else 2
                        while len(pending_pv) > depth:
                            emit_pv(*pending_pv.pop(0))
                        next(nxt, None)
                        next(nxt, None)
                # flush any remaining pipelined projection work
                for _ in nxt:
                    pass
            for args in pending_pv:
                emit_pv(*args)
    nc.compile()
    return nc


_NC_CACHE = None


def _get_nc():
    global _NC_CACHE
    if _NC_CACHE is None:
        _NC_CACHE = build_program()
    return _NC_CACHE


def _prep_inputs(hidden_states, attention_mask, head_mask, Wq, bq, Wk, bk, Wv, bv):
    hidden_states = np.asarray(hidden_states, dtype=np.float32)
    attention_mask = np.asarray(attention_mask, dtype=np.float32)
    head_mask = np.asarray(head_mask, dtype=np.float32)
    Wq = np.asarray(Wq, dtype=np.float32)
    bq = np.asarray(bq, dtype=np.float32)
    Wk = np.asarray(Wk, dtype=np.float32)
    bk = np.asarray(bk, dtype=np.float32)
    Wv = np.asarray(Wv, dtype=np.float32)
    bv = np.asarray(bv, dtype=np.float32)

    # fold head_mask into Wv/bv (probs*hm @ V == probs @ (hm*V))
    hm = head_mask.reshape(H)
    hscale = np.repeat(hm, DH).astype(np.float32)
    wqT = np.ascontiguousarray(Wq.T)
    wkT = np.ascontiguousarray(Wk.T)
    wvT = np.ascontiguousarray((Wv * hscale[:, None]).T)
    bq2d = np.ascontiguousarray(bq.reshape(KT, 128).T)
    bk2d = np.ascontiguousarray(bk.reshape(KT, 128).T)
    bvrow = (bv * hscale).reshape(1, D)

    mask = np.broadcast_to(
        attention_mask.reshape(attention_mask.shape[0], -1)[:, -S:], (N_CORES, S)
    )

    in_maps = []
    for b in range(N_CORES):
        in_maps.append(
            {
                "hT": np.ascontiguousarray(hidden_states[b].T),
                "wqT": wqT,
                "wkT": wkT,
                "wvT": wvT,
                "bq2d": bq2d,
                "bk2d": bk2d,
                "bvrow": bvrow,
                "mask2d": np.ascontiguousarray(mask[b].reshape(KT, 128).T),
            }
        )
    return in_maps


def _install_trace_shim():
    """antenv.axon_hooks is absent in this image; provide it so trace=True works."""
    import types

    if "antenv.axon_hooks" in sys.modules:
        return
    mod = types.ModuleType("antenv.axon_hooks")
    mod._hook = None

    def _set(h):
        mod._hook = h

    def _get():
        return mod._hook

    mod.set_axon_ntff_profile_hook = _set
    mod.get_axon_ntff_profile_hook = _get
    sys.modules["antenv.axon_hooks"] = mod
    try:
        from trn_agent_boot.trn_boot import _ntff_profile_via_ctypes

        _set(_ntff_profile_via_ctypes("/opt/axon/libaxon_pjrt.so"))
    except Exception:
        pass


def _kernel_impl(trace=False, **inputs):
    nc = _get_nc()
    in_maps = _prep_inputs(**inputs)
    kwargs = {}
    if trace:
        _install_trace_shim()
        kwargs["trace"] = True
        kwargs["trace_cores"] = list(range(N_CORES))
    res = run_bass_kernel_spmd(nc, in_maps, core_ids=list(range(N_CORES)), **kwargs)
    out = np.empty((N_CORES, S, D), dtype=np.float32)
    for b in range(N_CORES):
        out[b] = res.results[b]["ctxT"].T
    return out, res


def kernel(**inputs) -> np.ndarray:
    return _kernel_impl(trace=False, **inputs)[0]


# revision 32
# speedup vs baseline: 1.0206x; 1.0077x over previous
"""MoEBertSelfAttention on 8 Trainium2 NeuronCores.

Strategy: data-parallel over batch (B=8 -> one batch element per core).
Each core computes its element's full self-attention:
    q = h @ Wq.T + bq ; k, v likewise
    S = q_h k_h^T / sqrt(dh) + mask ; P = softmax(S) * head_mask
    ctx = P v_h, heads concatenated.

On-device dataflow is fully transposed to avoid any on-chip transposes:
  - host passes H^T and W^T; projections produce Q^T/K^T (feature-major)
    and V in normal layout (token-major),
  - scores are computed as S^T (key position on partitions) so the additive
    attention mask is a per-partition bias on the exp() activation,
  - the softmax denominator rides as an extra all-ones column of V in the
    PV matmul; normalization uses a batched reciprocal (reshaped to all 128
    partitions via a DRAM bounce) and a partition-broadcast DMA,
  - host transposes the returned ctx^T back.
head_mask is folded into Wv/bv on the host (exact: probs*hm @ V == probs @ (hm*V)).
Matmuls run in float32r (full PE rate); PSUM/softmax stay fp32.

The next head-pair's Q/K projection matmuls are software-pipelined into the
current pair's attention loop as PE filler work so the tensor engine never
idles while the activation engine runs exp().
"""

import sys

if "/opt/trn_rl_repo" not in sys.path:
    sys.path.insert(0, "/opt/trn_rl_repo")

import numpy as np

import concourse.bacc as bacc
import concourse.bass as bass
import concourse.tile as tile
from concourse import mybir
from concourse.bass_utils import run_bass_kernel_spmd

S = 1024  # sequence length
D = 1024  # hidden size
H = 16  # heads
DH = 64  # head size
KT = D // 128  # 128-row tiles along a feature dim
NT = S // 512  # 512-col tiles along the sequence
HP = H // 2  # head pairs
N_CORES = 8

F32 = mybir.dt.float32
F32R = mybir.dt.float32r


def _ts(i, n):
    return slice(i * n, (i + 1) * n)


def build_program():
    nc = bacc.Bacc("TRN2", target_bir_lowering=False, debug=False, num_devices=N_CORES)

    hT = nc.dram_tensor("hT", [D, S], F32R, kind="ExternalInput").ap()
    wqT = nc.dram_tensor("wqT", [D, D], F32R, kind="ExternalInput").ap()
    wkT = nc.dram_tensor("wkT", [D, D], F32R, kind="ExternalInput").ap()
    wvT = nc.dram_tensor("wvT", [D, D], F32R, kind="ExternalInput").ap()
    bq2d = nc.dram_tensor("bq2d", [128, KT], F32, kind="ExternalInput").ap()
    bk2d = nc.dram_tensor("bk2d", [128, KT], F32, kind="ExternalInput").ap()
    bvrow = nc.dram_tensor("bvrow", [1, D], F32, kind="ExternalInput").ap()
    mask2d = nc.dram_tensor("mask2d", [128, KT], F32, kind="ExternalInput").ap()
    ctxT = nc.dram_tensor("ctxT", [D, S], F32, kind="ExternalOutput").ap()
    # DRAM bounce buffers: rowsums out, reciprocals back (per head, flat 1024)
    rsums = nc.dram_tensor("rsums", [H, NT, 512], F32).ap()
    recips = nc.dram_tensor("recips", [H, NT, 512], F32).ap()

    hT_r = hT.rearrange("(kt p) s -> p kt s", p=128)
    wqT_r = wqT.rearrange("(kt p) o -> p kt o", p=128)
    wkT_r = wkT.rearrange("(kt p) o -> p kt o", p=128)
    wvT_r = wvT.rearrange("(kt p) o -> p kt o", p=128)

    with tile.TileContext(nc) as tc:
        with (
            tc.tile_pool(name="persist", bufs=1) as persist,
            tc.tile_pool(name="wpool", bufs=2) as wpool,
            tc.tile_pool(name="qkpool", bufs=2) as qkpool,
            tc.tile_pool(name="expool", bufs=8) as expool,
            tc.tile_pool(name="outpool", bufs=4) as outpool,
            tc.tile_pool(name="ps", bufs=2, space="PSUM") as ps,
        ):
            # ---- persistent SBUF ----
            # (first hT chunk + head pair 0's weights lead the DMA queues so
            # the first matmul can start within a few microseconds)
            hT_sb = persist.tile([128, KT, S], F32R)
            for kt in range(KT):
                for hh in range(2):
                    nc.sync.dma_start(
                        out=hT_sb[:, kt, _ts(hh, 512)], in_=hT_r[:, kt, _ts(hh, 512)]
                    )
            bq_sb = persist.tile([128, KT], F32)
            nc.sync.dma_start(out=bq_sb, in_=bq2d)
            bk_sb = persist.tile([128, KT], F32)
            nc.sync.dma_start(out=bk_sb, in_=bk2d)
            mask_sb = persist.tile([128, KT], F32)
            nc.sync.dma_start(out=mask_sb, in_=mask2d)
            # bv broadcast to all partitions (partition-step-0 DMA from DRAM)
            bv_bc = persist.tile([128, D], F32)
            nc.sync.dma_start(
                out=bv_bc,
                in_=bass.AP(tensor=bvrow.tensor, offset=0, ap=[[0, 128], [1, D]]),
            )
            ones_f = persist.tile([128, H], F32)
            nc.vector.memset(ones_f, 1.0)

            # V in token-major layout, one 65-wide block per head
            # ([64 cols of V_h | ones]); the ones column yields the softmax
            # denominator for free during the PV matmul.
            v_sb = persist.tile([128, KT, H * (DH + 1)], F32R)
            v4 = v_sb.rearrange("p st (h c) -> p st h c", c=DH + 1)
            for st in range(KT):
                nc.vector.tensor_copy(
                    v4[:, st, :, DH : DH + 1],
                    ones_f.rearrange("p (h o) -> p h o", o=1),
                )

            qk_tiles = {}

            def emit_qk(hp):
                """Q^T/K^T projection for head pair hp, yielded in small pieces
                so the caller can interleave them into attention emission."""
                wq_blk = wpool.tile([128, KT, 128], F32R, tag="wq", name=f"wq{hp}")
                nc.sync.dma_start(out=wq_blk, in_=wqT_r[:, :, _ts(hp, 128)])
                wk_blk = wpool.tile([128, KT, 128], F32R, tag="wk", name=f"wk{hp}")
                nc.sync.dma_start(out=wk_blk, in_=wkT_r[:, :, _ts(hp, 128)])
                res = []
                for pi, (blk, bias, tg) in enumerate(
                    ((wq_blk, bq_sb, "qT"), (wk_blk, bk_sb, "kT"))
                ):
                    t = qkpool.tile([128, S], F32R, tag=tg, name=f"{tg}{hp}")
                    for nt in range(NT):
                        p0 = ps.tile(
                            [128, 512], F32, tag="qk", bufs=1, name=f"pq{hp}_{pi}{nt}"
                        )
                        for kt in range(KT):
                            nc.tensor.matmul(
                                p0,
                                blk[:, kt, :],
                                hT_sb[:, kt, _ts(nt, 512)],
                                start=(kt == 0),
                                stop=(kt == KT - 1),
                            )
                            if kt % 2 == 1:
                                yield
                        nc.vector.tensor_scalar_add(
                            t[:, _ts(nt, 512)], p0, bias[:, hp : hp + 1]
                        )
                        yield
                    res.append(t)
                qk_tiles[hp] = res

            # head pair 0's projections up-front: weight DMAs + first matmuls
            # lead, then the remaining hT chunks, then the rest.
            for _ in emit_qk(0):
                pass

            # ---- V projection: V[s, o] = sum_d H^T[d, s] Wv^T[d, o] + bv[o] ----
            wvT_sb = persist.tile([128, KT, D], F32R)
            for kt in range(KT):
                for hh in range(2):
                    nc.sync.dma_start(
                        out=wvT_sb[:, kt, _ts(hh, 512)], in_=wvT_r[:, kt, _ts(hh, 512)]
                    )
            for st in range(KT):
                for nt in range(NT):
                    ps_v = ps.tile([128, 512], F32, tag="pv", bufs=3, name=f"psv{st}_{nt}")
                    for kt in range(KT):
                        nc.tensor.matmul(
                            ps_v,
                            hT_sb[:, kt, _ts(st, 128)],
                            wvT_sb[:, kt, _ts(nt, 512)],
                            start=(kt == 0),
                            stop=(kt == KT - 1),
                        )
                    # scatter into v_sb with the bias added on the way
                    nc.vector.tensor_tensor(
                        out=v4[:, st, 8 * nt : 8 * nt + 8, 0:DH],
                        in0=ps_v.rearrange("p (h c) -> p h c", c=DH),
                        in1=bv_bc[:, _ts(nt, 512)].rearrange("p (h c) -> p h c", c=DH),
                        op=mybir.AluOpType.add,
                    )

            # ---- attention, one-deep software pipeline over (pair, head, mt):
            # PV matmuls for unit n are emitted after unit n+1's scores+exp so
            # they never head-of-line-block the PE queue while exp(n) runs.
            pv_tiles = {}

            def emit_pv(hp, hl, mt, ex):
                h = 2 * hp + hl
                if mt == 0:
                    pv_tiles[h] = [
                        ps.tile(
                            [DH + 1, 512], F32, tag="pv", bufs=3, name=f"pspv{h}_{i}"
                        )
                        for i in range(NT)
                    ]
                for nt in range(NT):
                    nc.tensor.matmul(
                        pv_tiles[h][nt],
                        v_sb[:, mt, h * (DH + 1) : (h + 1) * (DH + 1)],
                        ex[:, _ts(nt, 512)],
                        start=(mt == 0),
                        stop=(mt == KT - 1),
                    )
                if mt == KT - 1:
                    emit_norm(h)

            def emit_norm(h):
                # rowsum row -> DRAM, batched reciprocal on [128, 8] (all
                # lanes), back to DRAM, partition-broadcast loads, final mul.
                ps_pv = pv_tiles[h]
                # copy ctx + rowsum out of PSUM first so the banks free quickly
                rs_sb = outpool.tile([DH + 1, NT, 512], F32, tag="rs", bufs=2, name=f"rs{h}")
                cs_sb = outpool.tile([DH, NT, 512], F32, tag="cs", bufs=2, name=f"cs{h}")
                for nt in range(NT):
                    nc.vector.tensor_copy(
                        rs_sb[DH : DH + 1, nt, :], ps_pv[nt][DH : DH + 1, :]
                    )
                    nc.vector.tensor_copy(cs_sb[:, nt, :], ps_pv[nt][0:DH, :])
                    nc.sync.dma_start(out=rsums[h, nt, :], in_=rs_sb[DH : DH + 1, nt, :])
                rc_sb = outpool.tile([128, KT], F32, tag="rc", bufs=2, name=f"rc{h}")
                nc.sync.dma_start(
                    out=rc_sb,
                    in_=bass.AP(tensor=rsums.tensor, offset=h * S, ap=[[KT, 128], [1, KT]]),
                )
                nc.vector.reciprocal(rc_sb, rc_sb)
                nc.sync.dma_start(
                    out=bass.AP(
                        tensor=recips.tensor, offset=h * S, ap=[[KT, 128], [1, KT]]
                    ),
                    in_=rc_sb,
                )
                for nt in range(NT):
                    bc_t = outpool.tile([DH, 512], F32, tag="bc", name=f"bc{h}_{nt}")
                    nc.sync.dma_start(
                        out=bc_t,
                        in_=bass.AP(
                            tensor=recips.tensor,
                            offset=h * S + nt * 512,
                            ap=[[0, DH], [1, 512]],
                        ),
                    )
                    stage = outpool.tile([DH, 512], F32, tag="stage", name=f"st{h}_{nt}")
                    nc.vector.tensor_mul(stage, cs_sb[:, nt, :], bc_t)
                    nc.sync.dma_start(
                        out=ctxT[h * DH : (h + 1) * DH, _ts(nt, 512)], in_=stage
                    )

            pending_pv = []
            for hp in range(HP):
                qT_t, kT_t = qk_tiles[hp]
                nxt = emit_qk(hp + 1) if hp + 1 < HP else iter(())
                for hl in range(2):
                    h = 2 * hp + hl
                    base = 64 * hl
                    for mt in range(KT):
                        # S^T[kpos, q] for this head
                        ps_s = ps.tile([128, 1024], F32, tag="sc", name=f"pss{h}_{mt}")
                        for nt in range(NT):
                            nc.tensor.matmul(
                                ps_s[:, _ts(nt, 512)],
                                kT_t[base : base + 64, _ts(mt, 128)],
                                qT_t[base : base + 64, _ts(nt, 512)],
                                start=True,
                                stop=True,
                            )
                        # probs_unnorm = exp(S^T/8 + mask[kpos])
                        ex = expool.tile([128, S], F32R, tag="ex", name=f"ex{h}_{mt}")
                        nc.scalar.activation(
                            ex,
                            ps_s,
                            mybir.ActivationFunctionType.Exp,
                            bias=mask_sb[:, mt : mt + 1],
                            scale=0.125,
                        )
                        pending_pv.append((hp, hl, mt, ex))
                        depth = 1 if (hp == HP - 1 and hl == 1) else 2
                        while len(pending_pv) > depth:
                            emit_pv(*pending_pv.pop(0))
                        next(nxt, None)
                        next(nxt, None)
                # flush any remaining pipelined projection work
                for _ in nxt:
                    pass
            for args in pending_pv:
                emit_pv(*args)
    nc.compile()
    return nc


_NC_CACHE = None


def _get_nc():
    global _NC_CACHE
    if _NC_CACHE is None:
        _NC_CACHE = build_program()
    return _NC_CACHE


def _prep_inputs(hidden_states, attention_mask, head_mask, Wq, bq, Wk, bk, Wv, bv):
    hidden_states = np.asarray(hidden_states, dtype=np.float32)
    attention_mask = np.asarray(attention_mask, dtype=np.float32)
    head_mask = np.asarray(head_mask, dtype=np.float32)
    Wq = np.asarray(Wq, dtype=np.float32)
    bq = np.asarray(bq, dtype=np.float32)
    Wk = np.asarray(Wk, dtype=np.float32)
    bk = np.asarray(bk, dtype=np.float32)
    Wv = np.asarray(Wv, dtype=np.float32)
    bv = np.asarray(bv, dtype=np.float32)

    # fold head_mask into Wv/bv (probs*hm @ V == probs @ (hm*V))
    hm = head_mask.reshape(H)
    hscale = np.repeat(hm, DH).astype(np.float32)
    wqT = np.ascontiguousarray(Wq.T)
    wkT = np.ascontiguousarray(Wk.T)
    wvT = np.ascontiguousarray((Wv * hscale[:, None]).T)
    bq2d = np.ascontiguousarray(bq.reshape(KT, 128).T)
    bk2d = np.ascontiguousarray(bk.reshape(KT, 128).T)
    bvrow = (bv * hscale).reshape(1, D)

    mask = np.broadcast_to(
        attention_mask.reshape(attention_mask.shape[0], -1)[:, -S:], (N_CORES, S)
    )

    in_maps = []
    for b in range(N_CORES):
        in_maps.append(
            {
                "hT": np.ascontiguousarray(hidden_states[b].T),
                "wqT": wqT,
                "wkT": wkT,
                "wvT": wvT,
                "bq2d": bq2d,
                "bk2d": bk2d,
                "bvrow": bvrow,
                "mask2d": np.ascontiguousarray(mask[b].reshape(KT, 128).T),
            }
        )
    return in_maps


def _install_trace_shim():
    """antenv.axon_hooks is absent in this image; provide it so trace=True works."""
    import types

    if "antenv.axon_hooks" in sys.modules:
        return
    mod = types.ModuleType("antenv.axon_hooks")
    mod._hook = None

    def _set(h):
        mod._hook = h

    def _get():
        return mod._hook

    mod.set_axon_ntff_profile_hook = _set
    mod.get_axon_ntff_profile_hook = _get
    sys.modules["antenv.axon_hooks"] = mod
    try:
        from trn_agent_boot.trn_boot import _ntff_profile_via_ctypes

        _set(_ntff_profile_via_ctypes("/opt/axon/libaxon_pjrt.so"))
    except Exception:
        pass


def _kernel_impl(trace=False, **inputs):
    nc = _get_nc()
    in_maps = _prep_inputs(**inputs)
    kwargs = {}
    if trace:
        _install_trace_shim()
        kwargs["trace"] = True
        kwargs["trace_cores"] = list(range(N_CORES))
    res = run_bass_kernel_spmd(nc, in_maps, core_ids=list(range(N_CORES)), **kwargs)
    out = np.empty((N_CORES, S, D), dtype=np.float32)
    for b in range(N_CORES):
        out[b] = res.results[b]["ctxT"].T
    return out, res


def kernel(**inputs) -> np.ndarray:
    return _kernel_impl(trace=False, **inputs)[0]


# revision 33
# speedup vs baseline: 1.0256x; 1.0050x over previous
"""MoEBertSelfAttention on 8 Trainium2 NeuronCores.

Strategy: data-parallel over batch (B=8 -> one batch element per core).
Each core computes its element's full self-attention:
    q = h @ Wq.T + bq ; k, v likewise
    S = q_h k_h^T / sqrt(dh) + mask ; P = softmax(S) * head_mask
    ctx = P v_h, heads concatenated.

On-device dataflow is fully transposed to avoid any on-chip transposes:
  - host passes H^T and W^T; projections produce Q^T/K^T (feature-major)
    and V in normal layout (token-major),
  - scores are computed as S^T (key position on partitions) so the additive
    attention mask is a per-partition bias on the exp() activation,
  - the softmax denominator rides as an extra all-ones column of V in the
    PV matmul; normalization uses a batched reciprocal (reshaped to all 128
    partitions via a DRAM bounce) and a partition-broadcast DMA,
  - host transposes the returned ctx^T back.
head_mask is folded into Wv/bv on the host (exact: probs*hm @ V == probs @ (hm*V)).
Matmuls run in float32r (full PE rate); PSUM/softmax stay fp32.

The next head-pair's Q/K projection matmuls are software-pipelined into the
current pair's attention loop as PE filler work so the tensor engine never
idles while the activation engine runs exp().
"""

import sys

if "/opt/trn_rl_repo" not in sys.path:
    sys.path.insert(0, "/opt/trn_rl_repo")

import numpy as np

import concourse.bacc as bacc
import concourse.bass as bass
import concourse.tile as tile
from concourse import mybir
from concourse.bass_utils import run_bass_kernel_spmd

S = 1024  # sequence length
D = 1024  # hidden size
H = 16  # heads
DH = 64  # head size
KT = D // 128  # 128-row tiles along a feature dim
NT = S // 512  # 512-col tiles along the sequence
HP = H // 2  # head pairs
N_CORES = 8

F32 = mybir.dt.float32
F32R = mybir.dt.float32r


def _ts(i, n):
    return slice(i * n, (i + 1) * n)


def build_program():
    nc = bacc.Bacc("TRN2", target_bir_lowering=False, debug=False, num_devices=N_CORES)

    hT = nc.dram_tensor("hT", [D, S], F32R, kind="ExternalInput").ap()
    wqT = nc.dram_tensor("wqT", [D, D], F32R, kind="ExternalInput").ap()
    wkT = nc.dram_tensor("wkT", [D, D], F32R, kind="ExternalInput").ap()
    wvT = nc.dram_tensor("wvT", [D, D], F32R, kind="ExternalInput").ap()
    bq2d = nc.dram_tensor("bq2d", [128, KT], F32, kind="ExternalInput").ap()
    bk2d = nc.dram_tensor("bk2d", [128, KT], F32, kind="ExternalInput").ap()
    bvrow = nc.dram_tensor("bvrow", [1, D], F32, kind="ExternalInput").ap()
    mask2d = nc.dram_tensor("mask2d", [128, KT], F32, kind="ExternalInput").ap()
    ctxT = nc.dram_tensor("ctxT", [D, S], F32, kind="ExternalOutput").ap()
    # DRAM bounce buffers: rowsums out, reciprocals back (per head, flat 1024)
    rsums = nc.dram_tensor("rsums", [H, NT, 512], F32).ap()
    recips = nc.dram_tensor("recips", [H, NT, 512], F32).ap()

    hT_r = hT.rearrange("(kt p) s -> p kt s", p=128)
    wqT_r = wqT.rearrange("(kt p) o -> p kt o", p=128)
    wkT_r = wkT.rearrange("(kt p) o -> p kt o", p=128)
    wvT_r = wvT.rearrange("(kt p) o -> p kt o", p=128)

    with tile.TileContext(nc) as tc:
        with (
            tc.tile_pool(name="persist", bufs=1) as persist,
            tc.tile_pool(name="wpool", bufs=2) as wpool,
            tc.tile_pool(name="qkpool", bufs=2) as qkpool,
            tc.tile_pool(name="expool", bufs=8) as expool,
            tc.tile_pool(name="outpool", bufs=4) as outpool,
            tc.tile_pool(name="ps", bufs=2, space="PSUM") as ps,
        ):
            # ---- persistent SBUF ----
            # (first hT chunk + head pair 0's weights lead the DMA queues so
            # the first matmul can start within a few microseconds)
            wq0_blk = wpool.tile([128, KT, 128], F32R, tag="wq", name="wq0")
            nc.sync.dma_start(out=wq0_blk, in_=wqT_r[:, :, _ts(0, 128)])
            wk0_blk = wpool.tile([128, KT, 128], F32R, tag="wk", name="wk0")
            nc.sync.dma_start(out=wk0_blk, in_=wkT_r[:, :, _ts(0, 128)])
            hT_sb = persist.tile([128, KT, S], F32R)
            for kt in range(KT):
                for hh in range(2):
                    nc.sync.dma_start(
                        out=hT_sb[:, kt, _ts(hh, 512)], in_=hT_r[:, kt, _ts(hh, 512)]
                    )
            bq_sb = persist.tile([128, KT], F32)
            nc.sync.dma_start(out=bq_sb, in_=bq2d)
            bk_sb = persist.tile([128, KT], F32)
            nc.sync.dma_start(out=bk_sb, in_=bk2d)
            mask_sb = persist.tile([128, KT], F32)
            nc.sync.dma_start(out=mask_sb, in_=mask2d)
            # bv broadcast to all partitions (partition-step-0 DMA from DRAM)
            bv_bc = persist.tile([128, D], F32)
            nc.sync.dma_start(
                out=bv_bc,
                in_=bass.AP(tensor=bvrow.tensor, offset=0, ap=[[0, 128], [1, D]]),
            )
            ones_f = persist.tile([128, H], F32)
            nc.vector.memset(ones_f, 1.0)

            # V in token-major layout, one 65-wide block per head
            # ([64 cols of V_h | ones]); the ones column yields the softmax
            # denominator for free during the PV matmul.
            v_sb = persist.tile([128, KT, H * (DH + 1)], F32R)
            v4 = v_sb.rearrange("p st (h c) -> p st h c", c=DH + 1)
            for st in range(KT):
                nc.vector.tensor_copy(
                    v4[:, st, :, DH : DH + 1],
                    ones_f.rearrange("p (h o) -> p h o", o=1),
                )

            qk_tiles = {}

            def emit_qk(hp, preloaded=None):
                """Q^T/K^T projection for head pair hp, yielded in small pieces
                so the caller can interleave them into attention emission."""
                if preloaded is not None:
                    wq_blk, wk_blk = preloaded
                else:
                    wq_blk = wpool.tile([128, KT, 128], F32R, tag="wq", name=f"wq{hp}")
                    nc.sync.dma_start(out=wq_blk, in_=wqT_r[:, :, _ts(hp, 128)])
                    wk_blk = wpool.tile([128, KT, 128], F32R, tag="wk", name=f"wk{hp}")
                    nc.sync.dma_start(out=wk_blk, in_=wkT_r[:, :, _ts(hp, 128)])
                res = []
                for pi, (blk, bias, tg) in enumerate(
                    ((wq_blk, bq_sb, "qT"), (wk_blk, bk_sb, "kT"))
                ):
                    t = qkpool.tile([128, S], F32R, tag=tg, name=f"{tg}{hp}")
                    for nt in range(NT):
                        p0 = ps.tile(
                            [128, 512], F32, tag="qk", bufs=1, name=f"pq{hp}_{pi}{nt}"
                        )
                        for kt in range(KT):
                            nc.tensor.matmul(
                                p0,
                                blk[:, kt, :],
                                hT_sb[:, kt, _ts(nt, 512)],
                                start=(kt == 0),
                                stop=(kt == KT - 1),
                            )
                            if kt % 2 == 1:
                                yield
                        nc.vector.tensor_scalar_add(
                            t[:, _ts(nt, 512)], p0, bias[:, hp : hp + 1]
                        )
                        yield
                    res.append(t)
                qk_tiles[hp] = res

            # head pair 0's projections up-front: weight DMAs + first matmuls
            # lead, then the remaining hT chunks, then the rest.
            for _ in emit_qk(0, preloaded=(wq0_blk, wk0_blk)):
                pass

            # ---- V projection: V[s, o] = sum_d H^T[d, s] Wv^T[d, o] + bv[o] ----
            wvT_sb = persist.tile([128, KT, D], F32R)
            for kt in range(KT):
                for hh in range(2):
                    nc.sync.dma_start(
                        out=wvT_sb[:, kt, _ts(hh, 512)], in_=wvT_r[:, kt, _ts(hh, 512)]
                    )
            for st in range(KT):
                for nt in range(NT):
                    ps_v = ps.tile([128, 512], F32, tag="pv", bufs=3, name=f"psv{st}_{nt}")
                    for kt in range(KT):
                        nc.tensor.matmul(
                            ps_v,
                            hT_sb[:, kt, _ts(st, 128)],
                            wvT_sb[:, kt, _ts(nt, 512)],
                            start=(kt == 0),
                            stop=(kt == KT - 1),
                        )
                    # scatter into v_sb with the bias added on the way
                    nc.vector.tensor_tensor(
                        out=v4[:, st, 8 * nt : 8 * nt + 8, 0:DH],
                        in0=ps_v.rearrange("p (h c) -> p h c", c=DH),
                        in1=bv_bc[:, _ts(nt, 512)].rearrange("p (h c) -> p h c", c=DH),
                        op=mybir.AluOpType.add,
                    )

            # ---- attention, one-deep software pipeline over (pair, head, mt):
            # PV matmuls for unit n are emitted after unit n+1's scores+exp so
            # they never head-of-line-block the PE queue while exp(n) runs.
            pv_tiles = {}

            def emit_pv(hp, hl, mt, ex):
                h = 2 * hp + hl
                if mt == 0:
                    pv_tiles[h] = [
                        ps.tile(
                            [DH + 1, 512], F32, tag="pv", bufs=3, name=f"pspv{h}_{i}"
                        )
                        for i in range(NT)
                    ]
                for nt in range(NT):
                    nc.tensor.matmul(
                        pv_tiles[h][nt],
                        v_sb[:, mt, h * (DH + 1) : (h + 1) * (DH + 1)],
                        ex[:, _ts(nt, 512)],
                        start=(mt == 0),
                        stop=(mt == KT - 1),
                    )
                if mt == KT - 1:
                    emit_norm(h)

            def emit_norm(h):
                # rowsum row -> DRAM, batched reciprocal on [128, 8] (all
                # lanes), back to DRAM, partition-broadcast loads, final mul.
                ps_pv = pv_tiles[h]
                # copy ctx + rowsum out of PSUM first so the banks free quickly
                rs_sb = outpool.tile([DH + 1, NT, 512], F32, tag="rs", bufs=2, name=f"rs{h}")
                cs_sb = outpool.tile([DH, NT, 512], F32, tag="cs", bufs=2, name=f"cs{h}")
                for nt in range(NT):
                    nc.vector.tensor_copy(
                        rs_sb[DH : DH + 1, nt, :], ps_pv[nt][DH : DH + 1, :]
                    )
                    nc.vector.tensor_copy(cs_sb[:, nt, :], ps_pv[nt][0:DH, :])
                    nc.sync.dma_start(out=rsums[h, nt, :], in_=rs_sb[DH : DH + 1, nt, :])
                rc_sb = outpool.tile([128, KT], F32, tag="rc", bufs=2, name=f"rc{h}")
                nc.sync.dma_start(
                    out=rc_sb,
                    in_=bass.AP(tensor=rsums.tensor, offset=h * S, ap=[[KT, 128], [1, KT]]),
                )
                nc.vector.reciprocal(rc_sb, rc_sb)
                nc.sync.dma_start(
                    out=bass.AP(
                        tensor=recips.tensor, offset=h * S, ap=[[KT, 128], [1, KT]]
                    ),
                    in_=rc_sb,
                )
                for nt in range(NT):
                    bc_t = outpool.tile([DH, 512], F32, tag="bc", name=f"bc{h}_{nt}")
                    nc.sync.dma_start(
                        out=bc_t,
                        in_=bass.AP(
                            tensor=recips.tensor,
                            offset=h * S + nt * 512,
                            ap=[[0, DH], [1, 512]],
                        ),
                    )
                    stage = outpool.tile([DH, 512], F32, tag="stage", name=f"st{h}_{nt}")
                    nc.vector.tensor_mul(stage, cs_sb[:, nt, :], bc_t)
                    nc.sync.dma_start(
                        out=ctxT[h * DH : (h + 1) * DH, _ts(nt, 512)], in_=stage
                    )

            pending_pv = []
            for hp in range(HP):
                qT_t, kT_t = qk_tiles[hp]
                nxt = emit_qk(hp + 1) if hp + 1 < HP else iter(())
                for hl in range(2):
                    h = 2 * hp + hl
                    base = 64 * hl
                    for mt in range(KT):
                        # S^T[kpos, q] for this head
                        ps_s = ps.tile([128, 1024], F32, tag="sc", name=f"pss{h}_{mt}")
                        for nt in range(NT):
                            nc.tensor.matmul(
                                ps_s[:, _ts(nt, 512)],
                                kT_t[base : base + 64, _ts(mt, 128)],
                                qT_t[base : base + 64, _ts(nt, 512)],
                                start=True,
                                stop=True,
                            )
                        # probs_unnorm = exp(S^T/8 + mask[kpos])
                        ex = expool.tile([128, S], F32R, tag="ex", name=f"ex{h}_{mt}")
                        nc.scalar.activation(
                            ex,
                            ps_s,
                            mybir.ActivationFunctionType.Exp,
                            bias=mask_sb[:, mt : mt + 1],
                            scale=0.125,
                        )
                        pending_pv.append((hp, hl, mt, ex))
                        depth = 1 if (hp == HP - 1 and hl == 1) else 2
                        while len(pending_pv) > depth:
                            emit_pv(*pending_pv.pop(0))
                        next(nxt, None)
                        next(nxt, None)
                # flush any remaining pipelined projection work
                for _ in nxt:
                    pass
            for args in pending_pv:
                emit_pv(*args)
    nc.compile()
    return nc


_NC_CACHE = None


def _get_nc():
    global _NC_CACHE
    if _NC_CACHE is None:
        _NC_CACHE = build_program()
    return _NC_CACHE


def _prep_inputs(hidden_states, attention_mask, head_mask, Wq, bq, Wk, bk, Wv, bv):
    hidden_states = np.asarray(hidden_states, dtype=np.float32)
    attention_mask = np.asarray(attention_mask, dtype=np.float32)
    head_mask = np.asarray(head_mask, dtype=np.float32)
    Wq = np.asarray(Wq, dtype=np.float32)
    bq = np.asarray(bq, dtype=np.float32)
    Wk = np.asarray(Wk, dtype=np.float32)
    bk = np.asarray(bk, dtype=np.float32)
    Wv = np.asarray(Wv, dtype=np.float32)
    bv = np.asarray(bv, dtype=np.float32)

    # fold head_mask into Wv/bv (probs*hm @ V == probs @ (hm*V))
    hm = head_mask.reshape(H)
    hscale = np.repeat(hm, DH).astype(np.float32)
    wqT = np.ascontiguousarray(Wq.T)
    wkT = np.ascontiguousarray(Wk.T)
    wvT = np.ascontiguousarray((Wv * hscale[:, None]).T)
    bq2d = np.ascontiguousarray(bq.reshape(KT, 128).T)
    bk2d = np.ascontiguousarray(bk.reshape(KT, 128).T)
    bvrow = (bv * hscale).reshape(1, D)

    mask = np.broadcast_to(
        attention_mask.reshape(attention_mask.shape[0], -1)[:, -S:], (N_CORES, S)
    )

    in_maps = []
    for b in range(N_CORES):
        in_maps.append(
            {
                "hT": np.ascontiguousarray(hidden_states[b].T),
                "wqT": wqT,
                "wkT": wkT,
                "wvT": wvT,
                "bq2d": bq2d,
                "bk2d": bk2d,
                "bvrow": bvrow,
                "mask2d": np.ascontiguousarray(mask[b].reshape(KT, 128).T),
            }
        )
    return in_maps


def _install_trace_shim():
    """antenv.axon_hooks is absent in this image; provide it so trace=True works."""
    import types

    if "antenv.axon_hooks" in sys.modules:
        return
    mod = types.ModuleType("antenv.axon_hooks")
    mod._hook = None

    def _set(h):
        mod._hook = h

    def _get():
        return mod._hook

    mod.set_axon_ntff_profile_hook = _set
    mod.get_axon_ntff_profile_hook = _get
    sys.modules["antenv.axon_hooks"] = mod
    try:
        from trn_agent_boot.trn_boot import _ntff_profile_via_ctypes

        _set(_ntff_profile_via_ctypes("/opt/axon/libaxon_pjrt.so"))
    except Exception:
        pass


def _kernel_impl(trace=False, **inputs):
    nc = _get_nc()
    in_maps = _prep_inputs(**inputs)
    kwargs = {}
    if trace:
        _install_trace_shim()
        kwargs["trace"] = True
        kwargs["trace_cores"] = list(range(N_CORES))
    res = run_bass_kernel_spmd(nc, in_maps, core_ids=list(range(N_CORES)), **kwargs)
    out = np.empty((N_CORES, S, D), dtype=np.float32)
    for b in range(N_CORES):
        out[b] = res.results[b]["ctxT"].T
    return out, res


def kernel(**inputs) -> np.ndarray:
    return _kernel_impl(trace=False, **inputs)[0]
